# revision 10
# baseline (speedup 1.0000x reference)
"""Cascaded attention cell (Bahdanau-attention RNN decoder) on 8 Trainium2 cores.

Data-parallel over batch: 16 batches per core, weights replicated.
Per-core Bass kernel precomputes UaH = x@Ua (SBUF-resident), XC = x@Co,
HU = inputs@Uo, EW = Emb@Wo, then runs the 96-step recurrence on-chip.
"""

import sys

for _p in ("/opt/trn_rl_repo",):
    if _p not in sys.path:
        sys.path.insert(0, _p)

import numpy as np

B, S, T, D, V = 128, 96, 256, 1024, 28
NCORES = 8
BC = B // NCORES  # 16 batches per core
KC = D // 128  # 8 contraction chunks
BIG = 1000.0

_nc_cache = {}


def build_nc(steps=S):
    """Build (and cache) the per-core Bass program."""
    if steps in _nc_cache:
        return _nc_cache[steps]

    from contextlib import ExitStack

    import concourse.bacc as bacc
    import concourse.mybir as mybir
    import concourse.tile as tile
    from concourse.masks import make_identity

    f32 = mybir.dt.float32
    f16 = mybir.dt.float16
    Tanh = mybir.ActivationFunctionType.Tanh
    Exp = mybir.ActivationFunctionType.Exp
    X = mybir.AxisListType.X
    op = mybir.AluOpType

    nc = bacc.Bacc("TRN2", target_bir_lowering=False, debug=False,
                   num_devices=NCORES)

    xT = nc.dram_tensor("xT", [BC, D, T], f32, kind="ExternalInput")
    hT = nc.dram_tensor("hT", [BC, D, steps], f32, kind="ExternalInput")
    y0T = nc.dram_tensor("y0T", [V, BC], f32, kind="ExternalInput")
    Ua = nc.dram_tensor("Ua", [D, D], f32, kind="ExternalInput")
    Wa = nc.dram_tensor("Wa", [V, D], f32, kind="ExternalInput")
    vaD = nc.dram_tensor("vaD", [D, BC, BC], f16, kind="ExternalInput")
    Uo = nc.dram_tensor("Uo", [D, V], f32, kind="ExternalInput")
    Co = nc.dram_tensor("Co", [D, V], f32, kind="ExternalInput")
    EmbT = nc.dram_tensor("EmbT", [V, V], f32, kind="ExternalInput")
    Wo = nc.dram_tensor("Wo", [V, V], f32, kind="ExternalInput")
    iota = nc.dram_tensor("iota", [BC, V], f32, kind="ExternalInput")
    iotaMB = nc.dram_tensor("iotaMB", [BC, V], f32, kind="ExternalInput")
    outT = nc.dram_tensor("outT", [V, steps, BC], f32, kind="ExternalOutput")

    with tile.TileContext(nc) as tc, \
         tc.tile_pool(name="persist", bufs=1) as persist:

        # Persistent SBUF tensors
        UaH_sb = persist.tile([128, KC, BC, T], f32)      # [e_in, e_chunk, b, t]
        XC_sb = persist.tile([128, 2, BC, V], f32)        # [t_in, t_chunk, b, v]
        HU_sb = persist.tile([V, steps, BC], f32)         # [v, s, b]
        ys_sb = persist.tile([V, steps, BC], f32)         # [v, s, b]
        Wa_sb = persist.tile([V, D], f32)
        vaD_sb = persist.tile([128, KC, BC, BC], f16)
        EW_sb = persist.tile([V, V], f32)
        iota_sb = persist.tile([BC, V], f32)
        iotaMB_sb = persist.tile([BC, V], f32)
        ident = persist.tile([128, 128], f32)
        y0T_sb = persist.tile([V, BC], f32)

        nc.sync.dma_start(out=Wa_sb, in_=Wa[:, :])
        nc.sync.dma_start(
            out=vaD_sb.rearrange("p k b m -> p k (b m)"),
            in_=vaD[:, :, :].rearrange("(k p) b m -> p k (b m)", p=128))
        nc.sync.dma_start(out=iota_sb, in_=iota[:, :])
        nc.sync.dma_start(out=iotaMB_sb, in_=iotaMB[:, :])
        nc.sync.dma_start(out=y0T_sb, in_=y0T[:, :])
        make_identity(nc, ident)

        # ---------------- precompute phase ----------------
        with tc.tile_pool(name="pc_w", bufs=1) as pcw, \
             tc.tile_pool(name="pc_x", bufs=2) as pcx, \
             tc.tile_pool(name="pc_ua", bufs=4) as pcu, \
             tc.tile_pool(name="pc_ps", bufs=2, space="PSUM") as pcp:

            embt_t = pcw.tile([V, V], f32)
            wo_t = pcw.tile([V, V], f32)
            uo_t = pcw.tile([128, KC, V], f32)
            co_t = pcw.tile([128, KC, V], f32)
            nc.sync.dma_start(out=embt_t, in_=EmbT[:, :])
            nc.sync.dma_start(out=wo_t, in_=Wo[:, :])
            nc.sync.dma_start(out=uo_t,
                              in_=Uo[:, :].rearrange("(k p) v -> p k v", p=128))
            nc.sync.dma_start(out=co_t,
                              in_=Co[:, :].rearrange("(k p) v -> p k v", p=128))

            ps_ew = pcp.tile([V, V], f32)
            nc.tensor.matmul(ps_ew, embt_t, wo_t, start=True, stop=True)
            nc.any.tensor_copy(EW_sb, ps_ew)

            for j in range(BC // 2):  # batch pairs
                xt = pcx.tile([128, KC, 2, T], f32)
                ht = pcx.tile([128, KC, 2, steps], f32)
                for bb in range(2):
                    nc.sync.dma_start(
                        out=xt[:, :, bb, :],
                        in_=xT[2 * j + bb, :, :].rearrange(
                            "(k p) t -> p k t", p=128))
                    nc.sync.dma_start(
                        out=ht[:, :, bb, :],
                        in_=hT[2 * j + bb, :, :].rearrange(
                            "(k p) s -> p k s", p=128))

                # UaH[:, m, 2j:2j+2, :] = (x_pair @ Ua[:, m-chunk])^T
                for m in range(KC):
                    ps = pcp.tile([128, 2 * T], f32)
                    for k in range(KC):
                        ua_t = pcu.tile([128, 128], f32)
                        nc.sync.dma_start(
                            out=ua_t,
                            in_=Ua[k * 128:(k + 1) * 128,
                                   m * 128:(m + 1) * 128])
                        nc.tensor.matmul(
                            ps, ua_t,
                            xt[:, k].rearrange("p b t -> p (b t)"),
                            start=(k == 0), stop=(k == KC - 1))
                    nc.any.tensor_copy(
                        UaH_sb[:, m, 2 * j:2 * j + 2, :].rearrange(
                            "p b t -> p (b t)"), ps)

                # XC for the pair
                for bb in range(2):
                    for tc2 in range(2):
                        psx = pcp.tile([128, V], f32)
                        for k in range(KC):
                            nc.tensor.matmul(
                                psx,
                                xt[:, k, bb, tc2 * 128:(tc2 + 1) * 128],
                                co_t[:, k, :],
                                start=(k == 0), stop=(k == KC - 1))
                        nc.any.tensor_copy(XC_sb[:, tc2, 2 * j + bb, :], psx)

                # HU for the pair
                psh = pcp.tile([V, 2, steps], f32)
                for k in range(KC):
                    nc.tensor.matmul(
                        psh.rearrange("p b s -> p (b s)"),
                        uo_t[:, k, :],
                        ht[:, k].rearrange("p b s -> p (b s)"),
                        start=(k == 0), stop=(k == KC - 1))
                nc.any.tensor_copy(
                    HU_sb[:, :, 2 * j:2 * j + 2].rearrange("p s b -> p b s"),
                    psh)

        # ---------------- scan phase ----------------
        with tc.tile_pool(name="sc_in", bufs=2) as scin, \
             tc.tile_pool(name="sc_out", bufs=2) as scout, \
             tc.tile_pool(name="sc_sm", bufs=2) as scsm, \
             tc.tile_pool(name="sc_ps", bufs=1, space="PSUM") as scps, \
             tc.tile_pool(name="sc_ps2", bufs=2, space="PSUM") as scps2:

            def argmax_onehot_T(yT_ap, tag):
                """yT (V, BC) -> one-hot^T (V, BC) of per-column argmax."""
                ps_yt = scps2.tile([BC, V], f32, tag="ps_yt")
                nc.tensor.transpose(ps_yt, yT_ap, ident[:V, :V])
                y_b = scsm.tile([BC, V], f32, tag="y_b")
                nc.vector.tensor_copy(y_b, ps_yt)
                mx = scsm.tile([BC, 1], f32, tag="mx")
                nc.vector.tensor_reduce(mx, y_b, axis=X, op=op.max)
                eq = scsm.tile([BC, V], f32, tag="eq")
                nc.vector.tensor_scalar(eq, y_b, mx, None, op0=op.is_equal)
                t1 = scsm.tile([BC, V], f32, tag="t1")
                nc.vector.tensor_mul(t1, eq, iotaMB_sb)
                t2 = scsm.tile([BC, V], f32, tag="t2")
                nc.vector.tensor_scalar(t2, t1, BIG, None, op0=op.add)
                amx = scsm.tile([BC, 1], f32, tag="amx")
                nc.vector.tensor_reduce(amx, t2, axis=X, op=op.min)
                oh = scsm.tile([BC, V], f32, tag="oh")
                nc.vector.tensor_scalar(oh, iota_sb, amx, None, op0=op.is_equal)
                ps_oh = scps2.tile([V, BC], f32, tag="ps_oh")
                nc.tensor.transpose(ps_oh, oh, ident[:BC, :BC])
                ohT = scsm.tile([V, BC], f32, tag="ohT")
                nc.vector.tensor_copy(ohT, ps_oh)
                return ohT

            ohT = argmax_onehot_T(y0T_sb, "boot")

            for s in range(steps):
                yT = y0T_sb if s == 0 else ys_sb[:, s - 1, :]

                # WaS^T (D on partitions), chunked
                ps_was = scps.tile([128, KC, BC], f32, tag="ps_was")
                for c in range(KC):
                    nc.tensor.matmul(
                        ps_was[:, c, :],
                        Wa_sb[:, c * 128:(c + 1) * 128], yT,
                        start=True, stop=True)
                was_sb = scsm.tile([128, KC, BC], f32, tag="was")
                nc.vector.tensor_copy(was_sb, ps_was)

                # tanh(UaH + WaS) and score reduction with va
                ps_sc = scps.tile([BC, T], f32, tag="ps_sc")
                for c in range(KC):
                    ti = scin.tile([128, BC, T], f16, tag="ti")
                    nc.vector.tensor_add(
                        ti, UaH_sb[:, c],
                        was_sb[:, c, :].unsqueeze(2).broadcast_to(
                            (128, BC, T)))
                    to = scout.tile([128, BC, T], f16, tag="to")
                    nc.scalar.activation(to, ti, Tanh)
                    for b in range(BC):
                        nc.tensor.matmul(
                            ps_sc, vaD_sb[:, c, b, :], to[:, b, :],
                            start=(c == 0 and b == 0),
                            stop=(c == KC - 1 and b == BC - 1),
                            skip_group_check=True)

                # softmax over T (rows = batches)
                negmax = scsm.tile([BC, 1], f32, tag="negmax")
                nc.vector.tensor_reduce(negmax, ps_sc, axis=X, op=op.max,
                                        negate=True)
                sm_e = scsm.tile([BC, T], f32, tag="sm_e")
                sumexp = scsm.tile([BC, 1], f32, tag="sumexp")
                nc.scalar.activation(sm_e, ps_sc, Exp, bias=negmax,
                                     accum_out=sumexp)
                rsum = scsm.tile([BC, 1], f32, tag="rsum")
                nc.vector.reciprocal(rsum, sumexp)
                sm_n = scsm.tile([BC, T], f32, tag="sm_n")
                nc.vector.tensor_scalar_mul(sm_n, sm_e, rsum)

                # transpose softmax -> (T, BC) in two 128-col halves
                ps_tr = scps.tile([128, 2, BC], f32, tag="ps_tr")
                for tc2 in range(2):
                    nc.tensor.transpose(
                        ps_tr[:, tc2, :],
                        sm_n[:, tc2 * 128:(tc2 + 1) * 128],
                        ident[:BC, :BC])
                smT = scsm.tile([128, 2, BC], f32, tag="smT")
                nc.vector.tensor_copy(smT, ps_tr)

                # y^T pre-activation: Emb[am]@Wo + ctx@Co (accumulated in PSUM)
                ps_y = scps.tile([V, BC], f32, tag="ps_y")
                nc.tensor.matmul(ps_y, EW_sb, ohT, start=True, stop=False,
                                 skip_group_check=True)
                for b in range(BC):
                    for tc2 in range(2):
                        nc.tensor.matmul(
                            ps_y[:, b:b + 1],
                            XC_sb[:, tc2, b, :], smT[:, tc2, b:b + 1],
                            start=False, stop=(tc2 == 1),
                            skip_group_check=True)

                # y = sigmoid(z) = 0.5 + 0.5*tanh(z/2), written to ys_sb
                z_sb = scsm.tile([V, BC], f32, tag="z_sb")
                nc.vector.tensor_add(z_sb, ps_y, HU_sb[:, s, :])
                th = scsm.tile([V, BC], f32, tag="th")
                nc.scalar.activation(th, z_sb, Tanh, scale=0.5)
                nc.vector.tensor_scalar(ys_sb[:, s, :], th, 0.5, 0.5,
                                        op0=op.mult, op1=op.add)

                if s + 1 < steps:
                    ohT = argmax_onehot_T(ys_sb[:, s, :], f"s{s}")

            nc.sync.dma_start(out=outT[:, :, :], in_=ys_sb)

    nc.compile()
    _nc_cache[steps] = nc
    return nc


def _make_vaD(va):
    """vaD[d, b, m] = va[d] if m == b else 0 (f16 lhsT for masked matvecs)."""
    vaD = np.zeros((D, BC, BC), np.float16)
    for b in range(BC):
        vaD[:, b, b] = va.astype(np.float16)
    return vaD


def make_in_maps(inputs, x, y0, Wa, Ua, Va, Wo, Uo, Co, Emb, steps=S):
    """Shard + lay out host-side inputs for the 8 cores."""
    f32 = np.float32
    inputs = np.asarray(inputs, f32)
    x = np.asarray(x, f32)
    y0 = np.asarray(y0, f32)
    shared = {
        "Ua": np.ascontiguousarray(np.asarray(Ua, f32)),
        "Wa": np.ascontiguousarray(np.asarray(Wa, f32)),
        "vaD": _make_vaD(np.asarray(Va, f32)[:, 0]),
        "Uo": np.ascontiguousarray(np.asarray(Uo, f32)),
        "Co": np.ascontiguousarray(np.asarray(Co, f32)),
        "EmbT": np.ascontiguousarray(np.asarray(Emb, f32).T),
        "Wo": np.ascontiguousarray(np.asarray(Wo, f32)),
        "iota": np.tile(np.arange(V, dtype=f32), (BC, 1)),
        "iotaMB": np.tile(np.arange(V, dtype=f32) - BIG, (BC, 1)),
    }
    in_maps = []
    for c in range(NCORES):
        sl = slice(c * BC, (c + 1) * BC)
        m = dict(shared)
        m["xT"] = np.ascontiguousarray(x[sl].transpose(0, 2, 1))
        m["hT"] = np.ascontiguousarray(
            inputs[sl, :steps, :].transpose(0, 2, 1))
        m["y0T"] = np.ascontiguousarray(y0[sl].T)
        in_maps.append(m)
    return in_maps


def gather_out(results, steps=S):
    out = np.empty((B, steps, V), np.float32)
    for c in range(NCORES):
        out[c * BC:(c + 1) * BC] = results[c]["outT"].transpose(2, 1, 0)
    return out


def kernel(inputs, x, y0, Wa, Ua, Va, Wo, Uo, Co, Emb):
    from concourse.bass_utils import run_bass_kernel_spmd

    nc = build_nc(S)
    in_maps = make_in_maps(inputs, x, y0, Wa, Ua, Va, Wo, Uo, Co, Emb, S)
    res = run_bass_kernel_spmd(nc, in_maps, list(range(NCORES)))
    return gather_out(res.results, S)
